# revision 14
# baseline (speedup 1.0000x reference)
"""Trainium2 Bass kernel for multi-head causal attention with RoPE.

Problem: x[4,2048,1024] -> MHA(16 heads, head_dim 64, RoPE, causal) -> [4,2048,1024]

Sharding: 8 cores = 4 batches x 2 head-groups (8 heads each, Megatron-style).
Each core computes a partial [T, C] projection output for its batch; the host
sums the two head-group partials per batch and adds b_proj.

Per-core dataflow (all on-device):
  - x^T via PE transposes
  - Q^T/K^T computed in [c', t] layout (head-pair tiles of 128 partitions),
    RoPE fused on the PSUM->SBUF path using host-precomputed cos/sin tables
    (1/sqrt(64) folded into W_q on host)
  - V in natural [t, c'] layout with a ones column per head (denominator trick)
  - scores S^T = K Q^T per (head, 512-wide q chunk, 128-wide k chunk) with
    causal block skipping; exp on ACT (max-subtraction-free softmax; scores
    are O(+-6) so exp is safe in fp32); diagonal blocks masked after exp
  - P@V in [q, d] form (lhsT = P^T slices) -> unnormalized O plus denominator
    column; normalized with per-partition reciprocal
  - y transposed back with PE; output projection accumulated over head pairs
"""

import math
import sys

import numpy as np

if "/opt/trn_rl_repo" not in sys.path:
    sys.path.insert(0, "/opt/trn_rl_repo")

import concourse.bass as bass
import concourse.tile as tile
from concourse import bacc
from concourse import mybir
from concourse.bass_utils import run_bass_kernel_spmd
from concourse.masks import make_identity

B, T, C = 4, 2048, 1024
NH, D = 16, 64
HL = 8              # local heads per core
DL = HL * D         # 512
NCORES = 8
P = 128
TCH = 512           # t-chunk width in phase A
NTC = T // TCH
ROPE_BASE = 10000.0

F32 = mybir.dt.float32
F32R = mybir.dt.float32r
BF16 = mybir.dt.bfloat16
Exp = mybir.ActivationFunctionType.Exp


def _emit(tc, xb, wqk, wv, wp, cos2, sin2, bias, mask, perm, out):
    nc = tc.nc
    with tc.tile_pool(name="pers", bufs=1) as pers:
        qkT = pers.tile([P, 8, T], F32R)          # j 0-3: Q pairs, 4-7: K pairs
        vsb = pers.tile([P, 16, HL * 65], BF16)   # [t mod 128, t tile, h*65 + e]
        ident = pers.tile([P, P], F32)
        make_identity(nc, ident)

        # ---------------- Phase A: x^T, Q^T/K^T (+RoPE), V ----------------
        with tc.tile_pool(name="pha", bufs=1) as pa, \
             tc.tile_pool(name="stage", bufs=2) as pstg, \
             tc.tile_pool(name="tmp", bufs=4) as ptmp, \
             tc.tile_pool(name="psA", bufs=2, space="PSUM") as psA, \
             tc.tile_pool(name="psq", bufs=2, space="PSUM") as psQ, \
             tc.tile_pool(name="psw", bufs=2, space="PSUM") as psW, \
             tc.tile_pool(name="psv", bufs=2, space="PSUM") as psV:
            wqk_sb = pa.tile([P, 8, 2 * DL], F32R)
            nc.gpsimd.dma_start(wqk_sb[:], wqk.rearrange("(o p) n -> p o n", p=P))
            wv_sb = pa.tile([P, 8, DL], F32R)
            nc.gpsimd.dma_start(wv_sb[:], wv.rearrange("(o p) n -> p o n", p=P))
            cos_sb = pa.tile([P, T], F32)
            nc.sync.dma_start(cos_sb[:], cos2)
            sin_sb = pa.tile([P, T], F32)
            nc.sync.dma_start(sin_sb[:], sin2)
            bias_sb = pa.tile([P, 8 + DL], F32)
            nc.sync.dma_start(bias_sb[:], bias)
            perm_sb = pa.tile([P, P], F32R)
            nc.gpsimd.dma_start(perm_sb[:], perm)

            for tcn in range(NTC):
                ts0 = tcn * TCH
                xT = pa.tile([P, 8, TCH], F32R, tag="xT")
                for i in range(TCH // P):
                    stg = pstg.tile([P, C], F32, tag="stg")
                    nc.sync.dma_start(stg[:], xb[ts0 + i * P: ts0 + (i + 1) * P, :])
                    for cc in range(8):
                        pst = psA.tile([P, P], F32)
                        nc.tensor.transpose(pst[:], stg[:, cc * P:(cc + 1) * P], ident[:])
                        nc.vector.tensor_copy(xT[:, cc, i * P:(i + 1) * P], pst[:])
                for j in range(8):
                    psq = psQ.tile([P, TCH], F32)
                    for cc in range(8):
                        nc.tensor.matmul(
                            psq[:],
                            wqk_sb[:, cc, j * P:(j + 1) * P],
                            xT[:, cc, :],
                            start=(cc == 0), stop=(cc == 7))
                    t1 = ptmp.tile([P, TCH], F32R, tag="t1")
                    nc.vector.tensor_scalar_add(t1[:], psq[:], bias_sb[:, j:j + 1])
                    psw = psW.tile([P, TCH], F32)
                    nc.tensor.matmul(psw[:], perm_sb[:], t1[:],
                                     start=True, stop=True)
                    dst = qkT[:, j, ts0:ts0 + TCH]
                    nc.vector.tensor_mul(dst, t1[:], cos_sb[:, ts0:ts0 + TCH])
                    swp = ptmp.tile([P, TCH], F32, tag="swp")
                    nc.vector.tensor_mul(swp[:], psw[:], sin_sb[:, ts0:ts0 + TCH])
                    nc.vector.tensor_add(dst, dst, swp[:])
                for i in range(TCH // P):
                    ti = tcn * (TCH // P) + i
                    psv = psV.tile([P, DL], F32)
                    for cc in range(8):
                        nc.tensor.matmul(
                            psv[:],
                            xT[:, cc, i * P:(i + 1) * P],
                            wv_sb[:, cc, :],
                            start=(cc == 0), stop=(cc == 7))
                    vv = vsb[:, ti].rearrange("p (h e) -> p h e", e=65)
                    nc.vector.tensor_tensor(
                        vv[:, :, 0:64],
                        psv.rearrange("p (h e) -> p h e", e=64),
                        bias_sb[:, 8:8 + DL].rearrange("p (h e) -> p h e", e=64),
                        mybir.AluOpType.add)
                    nc.vector.memset(vv[:, :, 64:65], 1.0)

        # ---------------- Phase B: attention ----------------
        with tc.tile_pool(name="phb", bufs=1) as pb:
            mask_sb = pb.tile([P, 4, 512], BF16)
            nc.sync.dma_start(mask_sb[:], mask)
            yT = pb.tile([P, 4, T], F32R)
            wp_sb = pb.tile([P, 4, C], F32R)
            nc.gpsimd.dma_start(wp_sb[:], wp.rearrange("(o p) n -> p o n", p=P))

            with tc.tile_pool(name="pT", bufs=18) as ppt, \
                 tc.tile_pool(name="ypair", bufs=2) as pyp, \
                 tc.tile_pool(name="rec", bufs=8) as prec, \
                 tc.tile_pool(name="psS", bufs=2, space="PSUM") as psS, \
                 tc.tile_pool(name="psO", bufs=4, space="PSUM") as psO:
                vg = vsb.rearrange("p a (h e) -> p a h e", e=65)
                for g in range(4):
                    ypair = pyp.tile([P, 16, P], F32)
                    for qc in range(4):
                        nkc = 4 * qc + 4
                        pts = []
                        for kc in range(nkc):
                            pss = psS.tile([P, 1024], F32, tag="psS")
                            for hh in range(2):
                                pb0 = hh * 64
                                nc.tensor.matmul(
                                    pss[:, hh * 512:(hh + 1) * 512],
                                    qkT[pb0:pb0 + 64, 4 + g, kc * P:(kc + 1) * P],
                                    qkT[pb0:pb0 + 64, g, qc * 512:(qc + 1) * 512],
                                    start=True, stop=True)
                            pt = ppt.tile([P, 1024], BF16, tag="pt")
                            nc.scalar.activation(pt[:], pss[:], Exp)
                            if kc >= 4 * qc:
                                m = kc - 4 * qc
                                nc.vector.tensor_mul(
                                    pt[:, 0:512], pt[:, 0:512], mask_sb[:, m])
                                nc.vector.tensor_mul(
                                    pt[:, 512:1024], pt[:, 512:1024], mask_sb[:, m])
                            pts.append(pt)
                        for hh in range(2):
                            for qs in range(4):
                                pso = psO.tile([P, 65], F32, tag="psO")
                                for kc in range(nkc):
                                    nc.tensor.matmul(
                                        pso[:],
                                        pts[kc][:, hh * 512 + qs * P:
                                                hh * 512 + (qs + 1) * P],
                                        vg[:, kc, 2 * g + hh],
                                        start=(kc == 0), stop=(kc == nkc - 1))
                                rec = prec.tile([P, 1], F32)
                                nc.vector.reciprocal(rec[:], pso[:, 64:65])
                                nc.vector.tensor_scalar_mul(
                                    ypair[:, qc * 4 + qs, hh * 64:(hh + 1) * 64],
                                    pso[:, 0:64], rec[:])
                    for ti in range(16):
                        psy = psS.tile([P, 1024], F32, tag="psS", name="psy")
                        nc.tensor.transpose(psy[:, 0:P], ypair[:, ti], ident[:])
                        nc.vector.tensor_copy(yT[:, g, ti * P:(ti + 1) * P],
                                              psy[:, 0:P])

            # ---------------- Phase C: output projection ----------------
            with tc.tile_pool(name="ost", bufs=3) as post, \
                 tc.tile_pool(name="psP", bufs=2, space="PSUM") as psP:
                for ti in range(16):
                    for n in range(2):
                        psp = psP.tile([P, 512], F32)
                        for g in range(4):
                            nc.tensor.matmul(
                                psp[:],
                                yT[:, g, ti * P:(ti + 1) * P],
                                wp_sb[:, g, n * 512:(n + 1) * 512],
                                start=(g == 0), stop=(g == 3))
                        ost = post.tile([P, 512], F32)
                        nc.vector.tensor_copy(ost[:], psp[:])
                        nc.sync.dma_start(
                            out[ti * P:(ti + 1) * P, n * 512:(n + 1) * 512], ost[:])


def build_nc():
    nc = bacc.Bacc("TRN2", target_bir_lowering=False, debug=False)
    xb = nc.dram_tensor("xb", [T, C], F32, kind="ExternalInput").ap()
    wqk = nc.dram_tensor("wqk", [C, 2 * DL], F32, kind="ExternalInput").ap()
    wv = nc.dram_tensor("wv", [C, DL], F32, kind="ExternalInput").ap()
    wp = nc.dram_tensor("wp", [DL, C], F32, kind="ExternalInput").ap()
    cos2 = nc.dram_tensor("cos2", [P, T], F32, kind="ExternalInput").ap()
    sin2 = nc.dram_tensor("sin2", [P, T], F32, kind="ExternalInput").ap()
    bias = nc.dram_tensor("bias", [P, 8 + DL], F32, kind="ExternalInput").ap()
    mask = nc.dram_tensor("mask", [P, 4, 512], BF16, kind="ExternalInput").ap()
    perm = nc.dram_tensor("perm", [P, P], F32, kind="ExternalInput").ap()
    out = nc.dram_tensor("out", [T, C], F32, kind="ExternalOutput").ap()
    with tile.TileContext(nc) as tc:
        _emit(tc, xb, wqk, wv, wp, cos2, sin2, bias, mask, perm, out)
    nc.compile()
    return nc


def rope_tables():
    inv_freq = 1.0 / (ROPE_BASE ** (np.arange(0, D, 2, dtype=np.float64) / D))
    t = np.arange(T, dtype=np.float64)
    freqs = np.outer(t, inv_freq)                      # [T, 32]
    emb = np.concatenate([freqs, freqs], axis=-1)      # [T, 64]
    cosT = np.cos(emb).T.astype(np.float32)            # [64, T]
    sinT = np.sin(emb).T.astype(np.float32)
    cos2 = np.tile(cosT, (2, 1)).copy()                # [128, T]
    sin2 = np.tile(sinT, (2, 1)).copy()
    return cos2, sin2


def perm_matrix():
    pm = np.zeros((P, P), dtype=np.float32)
    for base in (0, 64):
        for d in range(32):
            pm[base + d + 32, base + d] = -1.0       # rot_half: -x2 into top
            pm[base + d, base + d + 32] = 1.0        # +x1 into bottom
    return pm


def causal_masks():
    k = np.arange(P)[:, None]
    q = np.arange(512)[None, :]
    import ml_dtypes
    m = np.stack([(mm * P + k <= q) for mm in range(4)], axis=1)
    return np.ascontiguousarray(m.astype(ml_dtypes.bfloat16))  # [128, 4, 512]


def host_inputs(x, W_qkv, b_qkv, W_proj, b_proj):
    x = np.asarray(x, dtype=np.float32)
    W_qkv = np.asarray(W_qkv, dtype=np.float32)
    b_qkv = np.asarray(b_qkv, dtype=np.float32)
    W_proj = np.asarray(W_proj, dtype=np.float32)
    scale = 1.0 / math.sqrt(D)
    cos2, sin2 = rope_tables()
    masks = causal_masks()
    pm = perm_matrix()
    in_maps = []
    for core in range(NCORES):
        b = core // 2
        hg = core % 2
        s = hg * DL
        wq = W_qkv[:, s:s + DL] * scale
        wk = W_qkv[:, C + s:C + s + DL]
        wqk = np.ascontiguousarray(np.concatenate([wq, wk], axis=1))
        wv = np.ascontiguousarray(W_qkv[:, 2 * C + s:2 * C + s + DL])
        wp = np.ascontiguousarray(W_proj[s:s + DL, :])
        bq = b_qkv[s:s + DL] * scale
        bk = b_qkv[C + s:C + s + DL]
        bv = b_qkv[2 * C + s:2 * C + s + DL]
        bqk = np.concatenate([bq, bk]).reshape(8, P).T          # [128, 8]
        bvb = np.tile(bv[None, :], (P, 1))                      # [128, 512]
        bias = np.ascontiguousarray(
            np.concatenate([bqk, bvb], axis=1).astype(np.float32))
        in_maps.append({
            "xb": np.ascontiguousarray(x[b]),
            "wqk": wqk, "wv": wv, "wp": wp,
            "cos2": cos2, "sin2": sin2, "bias": bias, "mask": masks,
            "perm": pm,
        })
    return in_maps


_NC_CACHE = {}


def run(in_maps, **kwargs):
    if "nc" not in _NC_CACHE:
        _NC_CACHE["nc"] = build_nc()
    return run_bass_kernel_spmd(
        _NC_CACHE["nc"], in_maps, core_ids=list(range(NCORES)), **kwargs)


def kernel(x, W_qkv, b_qkv, W_proj, b_proj, **extra):
    in_maps = host_inputs(x, W_qkv, b_qkv, W_proj, b_proj)
    res = run(in_maps)
    b_proj = np.asarray(b_proj, dtype=np.float32)
    out = np.empty((B, T, C), dtype=np.float32)
    for b in range(B):
        out[b] = res.results[2 * b]["out"] + res.results[2 * b + 1]["out"] + b_proj
    return out
